# revision 16
# baseline (speedup 1.0000x reference)
"""Trainium2 Bass kernel: 16-head self-attention (B=2, N=2048, C=1024) on 8 cores.

Sharding: core c -> (batch b = c//4, head-group g = c%4 owning heads 4g..4g+3).
Each core computes QKV projection for its heads, full softmax attention, and a
partial out-projection (its heads' input-channel slice of W_out); the host sums
the 4 partials per batch (tensor-parallel all-reduce done on host at gather).

Performance structure (v2):
  - Phases are (head, query-half) with kt inner, so each softmax accumulator
    lives exactly one phase and its normalization overlaps the NEXT phase --
    the PE never idles longer than the ~3.4us HAM window, staying at 2.4 GHz.
  - exp() is the only ScalarE work (it is the steady-state bottleneck at
    ~1.11us per kt vs 0.86us of matmul); denominators are broadcast with a
    ones-matmul FIRST, then reciprocal'd wide ([64,1024]) on VectorE.
  - DMA order is arrival-paced: wqk/x chunks feed the first projection chain
    at ~9us; wo arrives last. All dram buffers are laid out so every DMA is
    contiguous on the dram side.
  - attn scale folded into W_q host-side; no exp shift (max exp(S) ~ 9e3 fits
    f32/bf16 comfortably); y partials returned as bf16.
  - out-projection for tokens 0:1024 is interleaved into the scalar-bound
    qh=1 phases; only tokens 1024:2048 remain for the tail.
"""
import os

import numpy as np

B, N, C, H, D = 2, 2048, 1024, 16, 64
HPC = 4            # heads per core
P = 128
SCALE = float(D) ** -0.5
KT = N // 128      # 16 key tiles

_cache = {}


def _build_nc():
    import concourse.bass as bass  # noqa: F401
    import concourse.mybir as mybir
    from concourse import bacc
    from concourse.tile import TileContext

    f32 = mybir.dt.float32
    f32r = mybir.dt.float32r
    bf16 = mybir.dt.bfloat16
    Exp = mybir.ActivationFunctionType.Exp
    mult = mybir.AluOpType.mult

    nc = bacc.Bacc("TRN2", target_bir_lowering=False, debug=False, num_devices=8)

    # dram layouts chosen so every DMA reads contiguous dram bytes
    xT = nc.dram_tensor("xT", [8, P, 8, 256], bf16, kind="ExternalInput")
    wqkA = nc.dram_tensor("wqkA", [P, 4, 512], bf16, kind="ExternalInput")
    wqkB = nc.dram_tensor("wqkB", [P, 4, 512], bf16, kind="ExternalInput")
    wv = nc.dram_tensor("wv", [P, 8, 256], bf16, kind="ExternalInput")
    wo = nc.dram_tensor("wo", [P, 2, 1024], bf16, kind="ExternalInput")
    bqk = nc.dram_tensor("bqk", [P, 4], f32, kind="ExternalInput")
    out_y = nc.dram_tensor("out_y", [N, C], bf16, kind="ExternalOutput")

    with TileContext(nc) as tc:
        with tc.tile_pool(name="sb", bufs=1) as sb, \
             tc.tile_pool(name="ps", bufs=1, space="PSUM") as ps:
            # ---- persistent SBUF tiles ----
            wqk_sb = sb.tile([P, 8, 512], bf16)
            wv_sb = sb.tile([P, 8, 256], bf16)
            wo_sb = sb.tile([P, 2, 1024], bf16)
            bqk_sb = sb.tile([P, 4], f32)
            xT_sb = sb.tile([P, 8, N], bf16)
            qkT_sb = sb.tile([P, 4, N], bf16)       # [q01|q23|k01|k23] x tokens
            v_sb = sb.tile([P, KT, HPC, 65], bf16)  # tokens x (head, D|ones)
            oT_sb = sb.tile([P, 2, N], bf16)        # head channels x q

            # ---- DMA issue order: few, large, arrival-paced transfers ----
            # (DMA-issue costs ~0.65us each serially on the sync queue)
            nc.sync.dma_start(wqk_sb[:, 0:4, :], wqkA[:])
            for ci in range(2):
                nc.sync.dma_start(xT_sb[:, :, ci * 256:(ci + 1) * 256],
                                  xT[ci, :, :, :])
            nc.sync.dma_start(bqk_sb, bqk[:])
            nc.sync.dma_start(wqk_sb[:, 4:8, :], wqkB[:])
            nc.sync.dma_start(wv_sb, wv[:])
            for ci in range(2, 8):
                nc.sync.dma_start(xT_sb[:, :, ci * 256:(ci + 1) * 256],
                                  xT[ci, :, :, :])
            nc.sync.dma_start(wo_sb, wo[:])

            ones_f = sb.tile([P, 1], f32)
            nc.vector.memset(ones_f, 1.0)
            with nc.allow_low_precision(reason="exact 1.0 to bf16"):
                nc.vector.tensor_copy(v_sb[:, :, :, 64:65],
                                      ones_f[:, 0:1, None].to_broadcast((P, KT, HPC, 1)))
            ones_r = sb.tile([1, 64], f32r)
            with nc.allow_low_precision(reason="exact 1.0 to f32r"):
                nc.vector.tensor_copy(ones_r, ones_f[0:1, :].to_broadcast((1, 64)))

            # ---- projection chains ----
            def qk_chain(ms, nt, half=None):
                if half is None:
                    tok = slice(nt * 512, (nt + 1) * 512)
                else:
                    tok = slice(nt * 512 + half * 256, nt * 512 + half * 256 + 256)
                w = tok.stop - tok.start
                qk_ps = ps.tile([P, 1024], f32, tag="s", bufs=2, name="qk_ps")[:, 0:w]
                for ks in range(8):
                    nc.tensor.matmul(
                        qk_ps,
                        lhsT=wqk_sb[:, ks, ms * 128:(ms + 1) * 128],
                        rhs=xT_sb[:, ks, tok],
                        start=(ks == 0), stop=(ks == 7),
                    )
                with nc.allow_low_precision(reason="qkT bf16 for PE"):
                    nc.vector.tensor_scalar_add(
                        qkT_sb[:, ms, tok], qk_ps, bqk_sb[:, ms:ms + 1])

            def v_chain(kt):
                v_ps = ps.tile([P, 1024], f32, tag="s", bufs=2, name="v_ps")[:, 0:256]
                for ks in range(8):
                    nc.tensor.matmul(
                        v_ps,
                        lhsT=xT_sb[:, ks, kt * 128:(kt + 1) * 128],
                        rhs=wv_sb[:, ks, :],
                        start=(ks == 0), stop=(ks == 7),
                    )
                with nc.allow_low_precision(reason="v bf16 for PE"):
                    nc.vector.tensor_copy(
                        v_sb[:, kt, :, 0:64],
                        v_ps.rearrange("p (h d) -> p h d", h=HPC))

            def chain(item):
                if item[0] == "v":
                    v_chain(item[1])
                else:
                    qk_chain(*item)

            # prelude: everything that only needs x chunks 0..3 (tokens 0:1024)
            prelude = [(0, 0, 0), (0, 0, 1), (2, 0), (1, 0), (3, 0), ("v", 0),
                       ("v", 1), ("v", 2), ("v", 3), (0, 1), (2, 1)]
            # jit chains keyed by (phase index, kt): v chains and k01 feed
            # phase 0; the remaining q/k chains spread across the otherwise
            # scalar-bound q0 phases, each finishing one phase ahead of its
            # first consumer.
            jit = {
                0: {1: [(1, 1)], 2: [(3, 1)], 3: [("v", 4)], 4: [(2, 2)],
                    5: [("v", 5)], 6: [("v", 6)], 7: [("v", 7)],
                    8: [("v", 8)], 9: [("v", 9)], 10: [("v", 10)],
                    11: [("v", 11), (2, 3)], 12: [("v", 12)], 13: [("v", 13)],
                    14: [("v", 14)], 15: [("v", 15)]},
                1: {3: [(3, 2)], 8: [(3, 3)]},
                2: {3: [(0, 2)], 8: [(0, 3)]},
                3: {3: [(1, 2)], 8: [(1, 3)]},
            }

            for item in prelude:
                chain(item)

            # ---- attention phases: (qh, head) outer, kt inner ----
            # pending normalizations: list of (o_ps tile, h, qh), drained at
            # the START of the following phase (overlapping its compute).
            pending = []
            # out-projection emission: first half (tokens 0:1024) interleaves
            # into the last three qh=1 phases; second half runs in the tail.
            def y_tile(qt):
                y_ps = ps.tile([P, 1024], f32, tag="s", bufs=2, name="y_ps")
                for n2 in range(2):
                    for ks2 in range(2):
                        nc.tensor.matmul(
                            y_ps[:, n2 * 512:(n2 + 1) * 512],
                            lhsT=oT_sb[:, ks2, qt * 128:(qt + 1) * 128],
                            rhs=wo_sb[:, ks2, n2 * 512:(n2 + 1) * 512],
                            start=(ks2 == 0), stop=(ks2 == 1),
                        )
                y_sb = sb.tile([P, 1024], bf16, tag="y", bufs=3, name="y_sb")
                with nc.allow_low_precision(reason="y partials bf16"):
                    nc.vector.tensor_copy(y_sb, y_ps)
                nc.sync.dma_start(out_y[qt * 128:(qt + 1) * 128, :], y_sb)

            def drain_norm():
                # normalization of the previous phase's accumulator; emitted
                # early in the current phase so it overlaps phase compute.
                o_t, h, qh = pending.pop(0)
                qsub, hp = h // 2, 64 * (h % 2)
                denom = sb.tile([1, 1024], f32r, tag="denom", bufs=2, name="denom")
                with nc.allow_low_precision(reason="softmax denom f32r"):
                    nc.vector.tensor_copy(denom, o_t[64:65, :])
                rb_ps = ps.tile([P, 1024], f32, tag="s", bufs=2, name="rb_ps")
                for j in range(2):
                    nc.tensor.matmul(
                        rb_ps[0:64, j * 512:(j + 1) * 512], lhsT=ones_r,
                        rhs=denom[0:1, j * 512:(j + 1) * 512],
                        start=True, stop=True,
                    )
                rbc = sb.tile([64, 1024], f32, tag="rbc", bufs=2, name="rbc")
                nc.vector.reciprocal_approx_fast(rbc, rb_ps[0:64, :])
                with nc.allow_low_precision(reason="oT bf16 for PE"):
                    nc.vector.tensor_tensor(
                        out=oT_sb[hp:hp + 64, qsub, qh * 1024:(qh + 1) * 1024],
                        in0=o_t[0:64, :],
                        in1=rbc,
                        op=mult,
                    )

            phases = [(h, 0) for h in range(HPC)] + [(h, 1) for h in range(HPC)]
            # y first-half tiles to interleave: qt 0..7
            y_first = list(range(8))
            for pi, (h, qh) in enumerate(phases):
                qsub, hp = h // 2, 64 * (h % 2)
                ksub = 2 + h // 2
                pjit = jit.get(pi, {})
                # 2 y tiles per qh=1 phase
                y_here = y_first[(pi - 4) * 2:(pi - 3) * 2] if pi >= 4 else []
                o_t = ps.tile([P, 1024], f32, tag="acc", bufs=2, name="o_ps")
                for kt in range(KT):
                    for item in pjit.get(kt, []):
                        chain(item)
                    if kt == 2 and pending:
                        drain_norm()
                    if kt in (5, 11) and y_here:
                        y_tile(y_here.pop(0))
                    key = slice(kt * 128, (kt + 1) * 128)
                    s_ps = ps.tile([P, 1024], f32, tag="s", bufs=2, name="s_ps")
                    for j in range(2):
                        qs = slice(qh * 1024 + j * 512, qh * 1024 + (j + 1) * 512)
                        nc.tensor.matmul(
                            s_ps[:, j * 512:(j + 1) * 512],
                            lhsT=qkT_sb[hp:hp + 64, ksub, key],
                            rhs=qkT_sb[hp:hp + 64, qsub, qs],
                            start=True, stop=True,
                        )
                    pT = sb.tile([P, 1024], bf16, tag="pT", bufs=4, name="pT")
                    nc.scalar.activation(pT, s_ps, Exp)
                    for j in range(2):
                        nc.tensor.matmul(
                            o_t[0:65, j * 512:(j + 1) * 512],
                            lhsT=v_sb[:, kt, h, :],
                            rhs=pT[:, j * 512:(j + 1) * 512],
                            start=(kt == 0), stop=(kt == KT - 1),
                        )
                pending.append((o_t, h, qh))

            # ---- tail: last phase's normalization + remaining out-proj ----
            drain_norm()
            for qt in range(8, 16):
                y_tile(qt)

    nc.compile()
    return nc


def _get_nc():
    if "nc" not in _cache:
        _cache["nc"] = _build_nc()
    return _cache["nc"]


def kernel(x, W_in, b_in, W_out, b_out):
    import ml_dtypes
    from concourse.bass_utils import run_bass_kernel_spmd

    _bf = ml_dtypes.bfloat16

    x = np.asarray(x, dtype=np.float32)
    W_in = np.asarray(W_in, dtype=np.float32)
    b_in = np.asarray(b_in, dtype=np.float32)
    W_out = np.asarray(W_out, dtype=np.float32)
    b_out = np.asarray(b_out, dtype=np.float32)

    in_maps = []
    for c in range(8):
        b, g = c // 4, c % 4
        rs = slice(256 * g, 256 * g + 256)

        # x[b].T -> [ks, p, ci, t] -> chunk-major [ci, p, ks, t]
        xTc = np.ascontiguousarray(
            x[b].T.reshape(8, 128, 8, 256).transpose(2, 1, 0, 3)).astype(_bf)
        # attn scale folded into the q rows (and q bias)
        Wq = W_in[0:C][rs] * SCALE
        Wk = W_in[C:2 * C][rs]
        Wqkc = np.concatenate([Wq, Wk])                            # [512,1024]
        wqkc = Wqkc.T.reshape(8, 128, 512).transpose(1, 0, 2).astype(_bf)
        wqkA_c = np.ascontiguousarray(wqkc[:, 0:4, :])
        wqkB_c = np.ascontiguousarray(wqkc[:, 4:8, :])
        Wv = W_in[2 * C:3 * C][rs]                                 # [256,1024]
        wvc = np.ascontiguousarray(
            Wv.T.reshape(8, 128, 256).transpose(1, 0, 2)).astype(_bf)
        WoT = np.ascontiguousarray(W_out[:, rs].T)                 # [256,1024]
        woc = np.ascontiguousarray(WoT.reshape(2, 128, 1024).transpose(1, 0, 2)).astype(_bf)
        bq = b_in[0:C][rs] * SCALE
        bk = b_in[C:2 * C][rs]
        bqkc = np.ascontiguousarray(
            np.concatenate([bq, bk]).reshape(4, 128).T)

        in_maps.append({"xT": xTc, "wqkA": wqkA_c, "wqkB": wqkB_c, "wv": wvc,
                        "wo": woc, "bqk": bqkc})

    nc = _get_nc()
    trace = os.environ.get("KERNEL_TRACE", "0") == "1"
    bkr = run_bass_kernel_spmd(nc, in_maps, core_ids=list(range(8)), trace=trace)
    _cache["last_bkr"] = bkr
    res = bkr.results

    y = np.zeros((B, N, C), dtype=np.float32)
    for c in range(8):
        y[c // 4] += res[c]["out_y"].astype(np.float32)
    # v-bias folds through softmax (rows sum to 1) and out-proj exactly
    y += (b_in[2 * C:3 * C] @ W_out.T + b_out)[None, None, :]
    return y
